# revision 39
# baseline (speedup 1.0000x reference)
"""Trainium2 Bass kernel for nn_AttentionMechanism_21646635172225.

Reference computation (per batch element n):
    q   = transpose(x[n], (T,C,H,W)).reshape(T, C*H*W)      # x[n]: (C,T,H,W)
    E   = q @ q.T                                            # (T, T)
    A   = softmax(E, axis=-1)
    out = alpha * (A @ q) + q          -> reshape/transpose back to (C,T,H,W)

Sharding: data-parallel over batch N=8 across the 8 NeuronCores (one batch
element per core), alpha replicated.

Per-core dataflow (C=128 on partitions, free axis = t*784 + hw):
  Phase 1, pipelined over nslot hw-striped chunks:
    - DMA the chunk of x into SBUF (XNQ, native layout, 784B runs).
    - GpSimd casts it to bf16 into a rotating chunk slot (XNbf).
    - TensorE accumulates the energy Gram matrix with 4-hw-packed bf16
      matmuls (128-column weights -> FWL weight loads) into PSUM P4; the
      packing leaves 4 diagonal 32x32 blocks to sum later.
    - VectorE 32x32 block-transposes the chunk into the "folded t-major"
      layout qt[32g+t, cl*stride + jj] = q[t, 32g+cl, hw].  The transpose of
      slot m writes slot m-1's (dead) region of XNQ, slot 0 a spare tail
      region, so no second full-size buffer exists.
    - ScalarE pre-casts the folded chunk to bf16 (qtb) for the phase-2
      matmuls (slot 3's casts are emitted after softmax to keep the ScalarE
      queue clear for it).
  Softmax: diagonal blocks of P4 are summed and replicated to the 4
    partition groups with accumulating selector matmuls; softmax runs on all
    128 lanes (Exp's accum_out provides the row sums); alpha is folded in
    (B = alpha*attn [+ I]); a 32x32 block transpose gives B^T per group.
  Phase 2, per slot: TensorE computes alpha*attn @ q (bf16, 4 concurrent
    32x32 tiles via tile_position); VectorE adds the exact fp32 residual
    from PSUM onto qt ("exact" mode; alpha=0 stays bitwise exact since
    0-weight matmuls produce exact zeros); slot halves DMA to HBM (y kept
    in the folded layout, de-folded on host).
"""

import sys

sys.path.insert(0, "/opt/trn_rl_repo")

from contextlib import ExitStack

import numpy as np

import concourse.bass as bass
import concourse.tile as tile
from concourse import bacc, mybir

# Problem shape (hardcoded per contract)
N, C, T, H, W = 8, 128, 32, 28, 28
HB = H * W  # 784
F = T * HB  # 25088
G = 4  # partition groups (c blocks of 32)
CL = 32  # c-local within group
NCORES = 8

f32 = mybir.dt.float32
bf16 = mybir.dt.bfloat16
AF = mybir.ActivationFunctionType
ALU = mybir.AluOpType
AX = mybir.AxisListType


def build_nc(
    mode: str = "exact",  # "exact" | "fused"
    nslot: int = 4,  # hw-striped chunks/slots (4 | HB/nslot required)
    nmm: int = 392,  # matmul2 moving free size
    cast_sub: int = 7,  # cast pieces per chunk (Js/cast_sub must be mult of epack)
    gs_num: int = 0,  # of every gs_den TT groups, this many go via GpSimd
    gs_den: int = 2,
    stores_per_slot: int = 2,
    epack: int = 4,  # hw columns per energy matmul (1 or 4)
    cast_engine: str = "scalar",  # engine for x->bf16 casts
    qtb_ahead: bool = True,  # pre-cast folded q to bf16 during phase 1
    qtb_gp_slots: tuple = (1,),  # qtb slots cast by GpSimd during phase 1
    qtb_late: int = 2,  # this many trailing slots' qtb cast after softmax
):
    assert HB % nslot == 0
    Js = HB // nslot  # hw per chunk/slot
    SW = Js * CL  # slot logical width
    assert SW % nmm == 0
    nk = SW // nmm  # mm chunks per slot
    assert nk % 4 == 0 or nk == 2
    kgrp = 4 if nk % 4 == 0 else 2  # psum banks per evac group
    assert CL % (2 * stores_per_slot) == 0
    assert Js % cast_sub == 0 and epack in (1, 4)

    nc = bacc.Bacc(trn_type="TRN2", target_bir_lowering=False, debug=False)

    x = nc.declare_dram_parameter("x", [C, F], f32, isOutput=False)
    al = nc.declare_dram_parameter("alpha_rep", [C, 1], f32, isOutput=False)
    sel4 = nc.declare_dram_parameter("sel4", [C, 4 * C], f32, isOutput=False)
    id32 = nc.declare_dram_parameter("ident32", [C, T], f32, isOutput=False)
    # y stored folded: host de-folds (see unfold_y)
    y = nc.declare_dram_parameter("y", [C, F], f32, isOutput=True)

    with ExitStack() as ctx:
        tc = ctx.enter_context(tile.TileContext(nc))
        consts = ctx.enter_context(tc.tile_pool(name="consts", bufs=1))
        smalls = ctx.enter_context(tc.tile_pool(name="smalls", bufs=1))
        xn_pool = ctx.enter_context(tc.tile_pool(name="xn", bufs=1))
        xnbf_pool = ctx.enter_context(tc.tile_pool(name="xnbf", bufs=2))
        qtb_pool = ctx.enter_context(
            tc.tile_pool(name="qtb", bufs=(nslot * nk) // kgrp)
        )
        psE_stack = ExitStack()
        psE = psE_stack.enter_context(tc.tile_pool(name="psE", bufs=1, space="PSUM"))

        alpha_sb = consts.tile([C, 1], f32)
        nc.sync.dma_start(alpha_sb[:], al[:])
        sel_sb = consts.tile([C, 4 * C], f32)
        nc.sync.dma_start(sel_sb[:], sel4[:])
        id_sb = consts.tile([C, T], f32)
        nc.sync.dma_start(id_sb[:], id32[:])
        # Warm the Exp activation table early (overlaps with phase-1 DMA).
        warm = consts.tile([C, 1], f32)
        nc.scalar.activation(warm[:], alpha_sb[:], AF.Exp)

        # XNQ = x (native) in cols [0, F) + one spare slot region at [F, F+SW)
        XNQ = xn_pool.tile([C, F + SW], f32)
        xn3 = XNQ[:, 0:F].rearrange("p (t h) -> p t h", t=T)
        xn_hwT = XNQ[:, 0:F].rearrange("p (t h) -> p h t", t=T)
        # x arrives slot-major-striped (host: make_in_maps) so every chunk
        # load reads a fully contiguous DRAM range at max HBM efficiency

        def qt_cells(m, cl0, ncl, j0, nj, jmajor=False):
            """AP over qt slot m cells: [p][cl][jj] (or [p][jj][cl])."""
            if m == 0:
                v = XNQ[:, F : F + SW].rearrange("p (cl j) -> p cl j", cl=CL)
                v = v[:, cl0 : cl0 + ncl, j0 : j0 + nj]
            else:
                base = (m - 1) * Js
                v = XNQ[:, 0:F].rearrange("p (cl h) -> p cl h", cl=CL)
                v = v[:, cl0 : cl0 + ncl, base + j0 : base + j0 + nj]
            if jmajor:
                v = v.rearrange("p cl j -> p j cl")
            return v

        cast_eng = {"gpsimd": nc.gpsimd, "scalar": nc.scalar, "vector": nc.vector}[
            cast_engine
        ]

        Bt = smalls.tile([C, T], f32)
        Btb = smalls.tile([C, T], bf16)
        qtbs = {}

        def emit_qtb(m, eng="scalar"):
            for k in range(nk // kgrp):
                qtb = qtb_pool.tile([C, kgrp * nmm], bf16, tag="qtb")
                qtbs[(m, k)] = qtb
                qb = qtb[:].rearrange(
                    "p (b cl2 j) -> p b cl2 j", b=kgrp, cl2=nmm // Js
                )
                src = qt_cells(
                    m, k * kgrp * (nmm // Js), kgrp * (nmm // Js), 0, Js
                ).rearrange("p (b cl2) j -> p b cl2 j", b=kgrp)
                if eng == "gpsimd":
                    nc.gpsimd.tensor_copy(qb, src)
                else:
                    nc.scalar.copy(qb, src)

        # ---- Phase 1: load + cast + energy + transpose-to-folded ----
        EP = T * epack
        P4 = psE.tile([EP, EP], f32)
        for m in range(nslot):
            sl = slice(m * Js, (m + 1) * Js)
            src = x[:, m * T * Js : (m + 1) * T * Js].rearrange(
                "p (t j) -> p t j", t=T
            )
            nc.sync.dma_start(xn3[:, :, sl], src)
            # slot layout: cell(t, j) = (j//ep)*(T*ep) + t*ep + j%ep, so each
            # energy group (all t, ep consecutive hw) is one contiguous
            # T*ep-column run (single-free-dim matmul weight AP, 256B reads)
            xb = xnbf_pool.tile([C, T * Js], bf16, tag="xnbf")
            ep = epack
            xb4 = xb[:].rearrange("p (jb t j4) -> p t jb j4", t=T, j4=ep)
            sub = Js // cast_sub
            assert sub % ep == 0
            for s in range(cast_sub):
                lo = s * sub
                hi = lo + sub
                o = xb4[:, :, lo // ep : hi // ep, :]
                i = xn3[:, :, m * Js + lo : m * Js + hi].rearrange(
                    "p t (jb j4) -> p t jb j4", j4=ep
                )
                if m == nslot - 1 and cast_engine == "gpsimd" and s >= cast_sub // 2:
                    nc.scalar.copy(o, i)  # split the last chunk's cast tail
                elif cast_engine == "scalar":
                    nc.scalar.copy(o, i)
                else:
                    cast_eng.tensor_copy(o, i)
            for jl in range(0, Js, ep):
                a = xb[:, (jl // ep) * T * ep : (jl // ep + 1) * T * ep]
                gidx = m * (Js // ep) + jl // ep
                nc.tensor.matmul(
                    P4[:],
                    a,
                    a,
                    start=(gidx == 0),
                    stop=(gidx == HB // ep - 1),
                )
            # transpose chunk m into qt slot m (region m-1 / spare)
            nc.vector.transpose(
                qt_cells(m, 0, CL, 0, Js, jmajor=True), xn_hwT[:, sl, :]
            )
            if qtb_ahead and m < nslot - qtb_late:
                emit_qtb(m, "gpsimd" if m in qtb_gp_slots else "scalar")

        # ---- Softmax -> B^T (replicated x4 on partition groups) ----
        P4sb = smalls.tile([EP, EP], f32)
        nc.scalar.copy(P4sb[:], P4[:])
        Erep = psE.tile([C, T], f32)
        if epack == 1:
            nc.tensor.matmul(Erep[:], sel_sb[0:T, 0:C], P4sb[:], start=True, stop=True)
        else:
            p4v = P4sb[:].rearrange("p (s j) -> p s j", j=epack)
            for jj in range(epack):
                nc.tensor.matmul(
                    Erep[:],
                    sel_sb[:, jj * C : (jj + 1) * C],
                    p4v[:, :, jj],
                    start=(jj == 0),
                    stop=(jj == epack - 1),
                )
        negmax = smalls.tile([C, 1], f32)
        nc.vector.tensor_reduce(
            negmax[:], Erep[:], axis=AX.X, op=ALU.max, negate=True
        )
        P = smalls.tile([C, T], f32)
        ssum = smalls.tile([C, 1], f32)
        nc.scalar.activation(
            P[:], Erep[:], AF.Exp, bias=negmax[:], scale=1.0, accum_out=ssum[:]
        )
        rcp = smalls.tile([C, 1], f32)
        nc.vector.reciprocal(rcp[:], ssum[:])
        Bp = smalls.tile([C, T], f32)
        nc.vector.tensor_scalar(
            out=Bp[:],
            in0=P[:],
            scalar1=rcp[:],
            scalar2=alpha_sb[:],
            op0=ALU.mult,
            op1=ALU.mult,
        )
        if mode == "fused":
            nc.vector.tensor_add(Bp[:], Bp[:], id_sb[:])
        nc.vector.transpose(Bt[:], Bp[:])
        nc.vector.tensor_copy(Btb[:], Bt[:])
        psE_stack.close()  # release P4/Erep PSUM banks for phase 2
        if qtb_ahead:
            for m in range(nslot - qtb_late, nslot):
                emit_qtb(m)

        # ---- Phase 2: attention matmul + residual + store ----
        # y is slot-major folded: y[p, m*SW + cl*Js + jj] -> every store
        # writes a contiguous DRAM range (host de-folds, see unfold_y)
        ncl_mm = nmm // Js
        with ExitStack() as p2:
            tmpp = (
                p2.enter_context(tc.tile_pool(name="tmp", bufs=2))
                if gs_num > 0
                else None
            )
            ps2 = p2.enter_context(tc.tile_pool(name="ps2", bufs=2, space="PSUM"))
            evac_idx = 0
            for m in range(nslot):
                if not qtb_ahead:
                    emit_qtb(m)
                for k in range(nk // kgrp):
                    qtb = qtbs[(m, k)]
                    ps = ps2.tile([C, kgrp * 512], f32)
                    for b in range(kgrp):
                        for g in range(G):
                            nc.tensor.matmul(
                                ps[g * 32 : (g + 1) * 32, b * 512 : b * 512 + nmm],
                                Btb[g * 32 : (g + 1) * 32, :],
                                qtb[g * 32 : (g + 1) * 32, b * nmm : (b + 1) * nmm],
                                start=True,
                                stop=True,
                                tile_position=(g * 32, g * 32),
                            )
                    pv = (
                        ps[:]
                        .rearrange("p (b r) -> p b r", b=kgrp)[:, :, 0:nmm]
                        .rearrange("p b (cl2 j) -> p b cl2 j", cl2=ncl_mm)
                    )
                    qv = qt_cells(
                        m, k * kgrp * ncl_mm, kgrp * ncl_mm, 0, Js
                    ).rearrange("p (b cl2) j -> p b cl2 j", b=kgrp)
                    if mode == "fused":
                        nc.scalar.copy(qv, pv)
                    else:
                        use_gp = (evac_idx % gs_den) < gs_num
                        evac_idx += 1
                        if use_gp:
                            tmp = tmpp.tile([C, kgrp * nmm], f32, tag="evac")
                            t3 = tmp[:].rearrange(
                                "p (b cl2 j) -> p b cl2 j", b=kgrp, cl2=ncl_mm
                            )
                            nc.scalar.copy(t3, pv)
                            nc.gpsimd.tensor_add(qv, qv, t3)
                        else:
                            nc.vector.tensor_add(qv, qv, pv)
                # store slot in pieces (cl ranges), contiguous in DRAM
                ncl_st = CL // stores_per_slot
                for s in range(stores_per_slot):
                    sb = qt_cells(m, s * ncl_st, ncl_st, 0, Js)
                    a = m * SW + s * ncl_st * Js
                    dr = y[:, a : a + ncl_st * Js].rearrange(
                        "p (cl j) -> p cl j", cl=ncl_st
                    )
                    nc.sync.dma_start(dr, sb)

    nc.compile()  # bacc passes: reg alloc, wait splitting (1-wait HW limit), ...
    return nc


def _consts():
    # sel4[u*4+jj', 32g+t] for block jj: 1 iff jj'==jj and u==t
    sel = np.zeros((C, 4 * C), np.float32)
    for jj in range(4):
        for t in range(T):
            for g in range(G):
                sel[t * 4 + jj, jj * C + g * 32 + t] = 1.0
    id32 = np.zeros((C, T), np.float32)
    for p in range(C):
        id32[p, p % T] = 1.0
    return sel, id32


_BUILD_KW = dict(mode="exact")


_NSLOT = 4  # must match build_nc(nslot=...)


def make_in_maps(x: np.ndarray, alpha: np.ndarray):
    assert x.shape == (N, C, T, H, W) and x.dtype == np.float32
    sel, id32 = _consts()
    alpha_rep = np.full((C, 1), np.float32(alpha.reshape(-1)[0]), np.float32)
    # slot-major stripe: x_str[p, m*T*Js + t*Js + j] = x[p, t, m*Js + j]
    Js = HB // _NSLOT
    xr = np.ascontiguousarray(
        x.reshape(N, C, T, _NSLOT, Js).transpose(0, 1, 3, 2, 4).reshape(N, C, F)
    )
    return [
        {"x": xr[n], "alpha_rep": alpha_rep, "sel4": sel, "ident32": id32}
        for n in range(NCORES)
    ]


def kernel(x: np.ndarray, alpha: np.ndarray) -> np.ndarray:
    from concourse.bass_utils import run_bass_kernel_spmd

    nc = build_nc(**_BUILD_KW)
    in_maps = make_in_maps(x, alpha)
    res = run_bass_kernel_spmd(nc, in_maps, list(range(NCORES)))
    out = np.stack([unfold_y(res.results[n]["y"]) for n in range(NCORES)])
    return out.astype(np.float32)


def unfold_y(yf: np.ndarray) -> np.ndarray:
    # yf[32g+t, m*SW + cl*Js + jj] = out[32g+cl, t, m*Js+jj]  ->  (C, T, H, W)
    Js = HB // _NSLOT
    return (
        np.asarray(yf)
        .reshape(G, T, _NSLOT, CL, Js)
        .transpose(0, 3, 1, 2, 4)
        .reshape(C, T, H, W)
    )


# revision 40
# speedup vs baseline: 1.2233x; 1.2233x over previous
"""Trainium2 Bass kernel for nn_AttentionMechanism_21646635172225.

Reference computation (per batch element n):
    q   = transpose(x[n], (T,C,H,W)).reshape(T, C*H*W)      # x[n]: (C,T,H,W)
    E   = q @ q.T                                            # (T, T)
    A   = softmax(E, axis=-1)
    out = alpha * (A @ q) + q          -> reshape/transpose back to (C,T,H,W)

Sharding: data-parallel over batch N=8 across the 8 NeuronCores (one batch
element per core), alpha replicated.

Per-core dataflow (C=128 on partitions, free axis = t*784 + hw):
  Phase 1, pipelined over nslot hw-striped chunks:
    - DMA the chunk of x into SBUF (XNQ, native layout, 784B runs).
    - GpSimd casts it to bf16 into a rotating chunk slot (XNbf).
    - TensorE accumulates the energy Gram matrix with 4-hw-packed bf16
      matmuls (128-column weights -> FWL weight loads) into PSUM P4; the
      packing leaves 4 diagonal 32x32 blocks to sum later.
    - VectorE 32x32 block-transposes the chunk into the "folded t-major"
      layout qt[32g+t, cl*stride + jj] = q[t, 32g+cl, hw].  The transpose of
      slot m writes slot m-1's (dead) region of XNQ, slot 0 a spare tail
      region, so no second full-size buffer exists.
    - ScalarE pre-casts the folded chunk to bf16 (qtb) for the phase-2
      matmuls (slot 3's casts are emitted after softmax to keep the ScalarE
      queue clear for it).
  Softmax: diagonal blocks of P4 are summed and replicated to the 4
    partition groups with accumulating selector matmuls; softmax runs on all
    128 lanes (Exp's accum_out provides the row sums); alpha is folded in
    (B = alpha*attn [+ I]); a 32x32 block transpose gives B^T per group.
  Phase 2, per slot: TensorE computes alpha*attn @ q (bf16, 4 concurrent
    32x32 tiles via tile_position); VectorE adds the exact fp32 residual
    from PSUM onto qt ("exact" mode; alpha=0 stays bitwise exact since
    0-weight matmuls produce exact zeros); slot halves DMA to HBM (y kept
    in the folded layout, de-folded on host).
"""

import sys

sys.path.insert(0, "/opt/trn_rl_repo")

from contextlib import ExitStack

import numpy as np

import concourse.bass as bass
import concourse.tile as tile
from concourse import bacc, mybir

# Problem shape (hardcoded per contract)
N, C, T, H, W = 8, 128, 32, 28, 28
HB = H * W  # 784
F = T * HB  # 25088
G = 4  # partition groups (c blocks of 32)
CL = 32  # c-local within group
NCORES = 8

f32 = mybir.dt.float32
bf16 = mybir.dt.bfloat16
AF = mybir.ActivationFunctionType
ALU = mybir.AluOpType
AX = mybir.AxisListType


def build_nc(
    mode: str = "exact",  # "exact" | "fused"
    nslot: int = 4,  # hw-striped chunks/slots (4 | HB/nslot required)
    nmm: int = 392,  # matmul2 moving free size
    cast_sub: int = 7,  # cast pieces per chunk (Js/cast_sub must be mult of epack)
    gs_num: int = 0,  # of every gs_den TT groups, this many go via GpSimd
    gs_den: int = 2,
    stores_per_slot: int = 2,
    epack: int = 4,  # hw columns per energy matmul (1 or 4)
    cast_engine: str = "scalar",  # engine for x->bf16 casts
    qtb_ahead: bool = False,  # pre-cast folded q to bf16 during phase 1
    qtb_gp_slots: tuple = (),  # qtb slots cast by GpSimd during phase 1
    qtb_late: int = 2,  # this many trailing slots' qtb cast after softmax
):
    assert HB % nslot == 0
    Js = HB // nslot  # hw per chunk/slot
    SW = Js * CL  # slot logical width
    assert SW % nmm == 0
    nk = SW // nmm  # mm chunks per slot
    assert nk % 4 == 0 or nk == 2
    kgrp = 4 if nk % 4 == 0 else 2  # psum banks per evac group
    assert CL % (2 * stores_per_slot) == 0
    assert Js % cast_sub == 0 and epack in (1, 4)

    nc = bacc.Bacc(trn_type="TRN2", target_bir_lowering=False, debug=False)

    x = nc.declare_dram_parameter("x", [C, F], f32, isOutput=False)
    al = nc.declare_dram_parameter("alpha_rep", [C, 1], f32, isOutput=False)
    sel4 = nc.declare_dram_parameter("sel4", [C, 4 * C], f32, isOutput=False)
    id32 = nc.declare_dram_parameter("ident32", [C, T], f32, isOutput=False)
    # y stored folded: host de-folds (see unfold_y)
    y = nc.declare_dram_parameter("y", [C, F], f32, isOutput=True)

    with ExitStack() as ctx:
        tc = ctx.enter_context(tile.TileContext(nc))
        consts = ctx.enter_context(tc.tile_pool(name="consts", bufs=1))
        smalls = ctx.enter_context(tc.tile_pool(name="smalls", bufs=1))
        xn_pool = ctx.enter_context(tc.tile_pool(name="xn", bufs=1))
        xnbf_pool = ctx.enter_context(tc.tile_pool(name="xnbf", bufs=2))
        qtb_pool = ctx.enter_context(
            tc.tile_pool(name="qtb", bufs=(nslot * nk) // kgrp)
        )
        psE_stack = ExitStack()
        psE = psE_stack.enter_context(tc.tile_pool(name="psE", bufs=1, space="PSUM"))

        alpha_sb = consts.tile([C, 1], f32)
        nc.sync.dma_start(alpha_sb[:], al[:])
        sel_sb = consts.tile([C, 4 * C], f32)
        nc.sync.dma_start(sel_sb[:], sel4[:])
        id_sb = consts.tile([C, T], f32)
        nc.sync.dma_start(id_sb[:], id32[:])
        # Warm the Exp activation table early (overlaps with phase-1 DMA).
        warm = consts.tile([C, 1], f32)
        nc.scalar.activation(warm[:], alpha_sb[:], AF.Exp)

        # XNQ = x (native) in cols [0, F) + one spare slot region at [F, F+SW)
        XNQ = xn_pool.tile([C, F + SW], f32)
        xn3 = XNQ[:, 0:F].rearrange("p (t h) -> p t h", t=T)
        xn_hwT = XNQ[:, 0:F].rearrange("p (t h) -> p h t", t=T)
        # x arrives slot-major-striped (host: make_in_maps) so every chunk
        # load reads a fully contiguous DRAM range at max HBM efficiency

        def qt_cells(m, cl0, ncl, j0, nj, jmajor=False):
            """AP over qt slot m cells: [p][cl][jj] (or [p][jj][cl])."""
            if m == 0:
                v = XNQ[:, F : F + SW].rearrange("p (cl j) -> p cl j", cl=CL)
                v = v[:, cl0 : cl0 + ncl, j0 : j0 + nj]
            else:
                base = (m - 1) * Js
                v = XNQ[:, 0:F].rearrange("p (cl h) -> p cl h", cl=CL)
                v = v[:, cl0 : cl0 + ncl, base + j0 : base + j0 + nj]
            if jmajor:
                v = v.rearrange("p cl j -> p j cl")
            return v

        cast_eng = {"gpsimd": nc.gpsimd, "scalar": nc.scalar, "vector": nc.vector}[
            cast_engine
        ]

        Bt = smalls.tile([C, T], f32)
        Btb = smalls.tile([C, T], bf16)
        qtbs = {}

        def emit_qtb(m, eng="scalar"):
            for k in range(nk // kgrp):
                qtb = qtb_pool.tile([C, kgrp * nmm], bf16, tag="qtb")
                qtbs[(m, k)] = qtb
                qb = qtb[:].rearrange(
                    "p (b cl2 j) -> p b cl2 j", b=kgrp, cl2=nmm // Js
                )
                src = qt_cells(
                    m, k * kgrp * (nmm // Js), kgrp * (nmm // Js), 0, Js
                ).rearrange("p (b cl2) j -> p b cl2 j", b=kgrp)
                if eng == "gpsimd":
                    nc.gpsimd.tensor_copy(qb, src)
                else:
                    nc.scalar.copy(qb, src)

        # ---- Phase 1: load + cast + energy + transpose-to-folded ----
        EP = T * epack
        P4 = psE.tile([EP, EP], f32)
        for m in range(nslot):
            sl = slice(m * Js, (m + 1) * Js)
            src = x[:, m * T * Js : (m + 1) * T * Js].rearrange(
                "p (t j) -> p t j", t=T
            )
            nc.sync.dma_start(xn3[:, :, sl], src)
            # slot layout: cell(t, j) = (j//ep)*(T*ep) + t*ep + j%ep, so each
            # energy group (all t, ep consecutive hw) is one contiguous
            # T*ep-column run (single-free-dim matmul weight AP, 256B reads)
            xb = xnbf_pool.tile([C, T * Js], bf16, tag="xnbf")
            ep = epack
            xb4 = xb[:].rearrange("p (jb t j4) -> p t jb j4", t=T, j4=ep)
            sub = Js // cast_sub
            assert sub % ep == 0
            for s in range(cast_sub):
                lo = s * sub
                hi = lo + sub
                o = xb4[:, :, lo // ep : hi // ep, :]
                i = xn3[:, :, m * Js + lo : m * Js + hi].rearrange(
                    "p t (jb j4) -> p t jb j4", j4=ep
                )
                if m == nslot - 1 and cast_engine == "gpsimd" and s >= cast_sub // 2:
                    nc.scalar.copy(o, i)  # split the last chunk's cast tail
                elif cast_engine == "scalar":
                    nc.scalar.copy(o, i)
                else:
                    cast_eng.tensor_copy(o, i)
            for jl in range(0, Js, ep):
                a = xb[:, (jl // ep) * T * ep : (jl // ep + 1) * T * ep]
                gidx = m * (Js // ep) + jl // ep
                nc.tensor.matmul(
                    P4[:],
                    a,
                    a,
                    start=(gidx == 0),
                    stop=(gidx == HB // ep - 1),
                )
            # transpose chunk m into qt slot m (region m-1 / spare)
            nc.vector.transpose(
                qt_cells(m, 0, CL, 0, Js, jmajor=True), xn_hwT[:, sl, :]
            )
            if qtb_ahead and m < nslot - qtb_late:
                emit_qtb(m, "gpsimd" if m in qtb_gp_slots else "scalar")

        # ---- Softmax -> B^T (replicated x4 on partition groups) ----
        P4sb = smalls.tile([EP, EP], f32)
        nc.scalar.copy(P4sb[:], P4[:])
        Erep = psE.tile([C, T], f32)
        if epack == 1:
            nc.tensor.matmul(Erep[:], sel_sb[0:T, 0:C], P4sb[:], start=True, stop=True)
        else:
            p4v = P4sb[:].rearrange("p (s j) -> p s j", j=epack)
            for jj in range(epack):
                nc.tensor.matmul(
                    Erep[:],
                    sel_sb[:, jj * C : (jj + 1) * C],
                    p4v[:, :, jj],
                    start=(jj == 0),
                    stop=(jj == epack - 1),
                )
        negmax = smalls.tile([C, 1], f32)
        nc.vector.tensor_reduce(
            negmax[:], Erep[:], axis=AX.X, op=ALU.max, negate=True
        )
        P = smalls.tile([C, T], f32)
        ssum = smalls.tile([C, 1], f32)
        nc.scalar.activation(
            P[:], Erep[:], AF.Exp, bias=negmax[:], scale=1.0, accum_out=ssum[:]
        )
        rcp = smalls.tile([C, 1], f32)
        nc.vector.reciprocal(rcp[:], ssum[:])
        Bp = smalls.tile([C, T], f32)
        nc.vector.tensor_scalar(
            out=Bp[:],
            in0=P[:],
            scalar1=rcp[:],
            scalar2=alpha_sb[:],
            op0=ALU.mult,
            op1=ALU.mult,
        )
        if mode == "fused":
            nc.vector.tensor_add(Bp[:], Bp[:], id_sb[:])
        nc.vector.transpose(Bt[:], Bp[:])
        nc.vector.tensor_copy(Btb[:], Bt[:])
        psE_stack.close()  # release P4/Erep PSUM banks for phase 2
        if qtb_ahead:
            for m in range(nslot - qtb_late, nslot):
                emit_qtb(m)

        # ---- Phase 2: attention matmul + residual + store ----
        # y is slot-major folded: y[p, m*SW + cl*Js + jj] -> every store
        # writes a contiguous DRAM range (host de-folds, see unfold_y)
        ncl_mm = nmm // Js
        with ExitStack() as p2:
            tmpp = (
                p2.enter_context(tc.tile_pool(name="tmp", bufs=2))
                if gs_num > 0
                else None
            )
            ps2 = p2.enter_context(tc.tile_pool(name="ps2", bufs=2, space="PSUM"))
            evac_idx = 0
            for m in range(nslot):
                if not qtb_ahead:
                    emit_qtb(m)
                for k in range(nk // kgrp):
                    qtb = qtbs[(m, k)]
                    ps = ps2.tile([C, kgrp * 512], f32)
                    for b in range(kgrp):
                        for g in range(G):
                            nc.tensor.matmul(
                                ps[g * 32 : (g + 1) * 32, b * 512 : b * 512 + nmm],
                                Btb[g * 32 : (g + 1) * 32, :],
                                qtb[g * 32 : (g + 1) * 32, b * nmm : (b + 1) * nmm],
                                start=True,
                                stop=True,
                                tile_position=(g * 32, g * 32),
                            )
                    pv = (
                        ps[:]
                        .rearrange("p (b r) -> p b r", b=kgrp)[:, :, 0:nmm]
                        .rearrange("p b (cl2 j) -> p b cl2 j", cl2=ncl_mm)
                    )
                    qv = qt_cells(
                        m, k * kgrp * ncl_mm, kgrp * ncl_mm, 0, Js
                    ).rearrange("p (b cl2) j -> p b cl2 j", b=kgrp)
                    if mode == "fused":
                        nc.scalar.copy(qv, pv)
                    else:
                        use_gp = (evac_idx % gs_den) < gs_num
                        evac_idx += 1
                        if use_gp:
                            tmp = tmpp.tile([C, kgrp * nmm], f32, tag="evac")
                            t3 = tmp[:].rearrange(
                                "p (b cl2 j) -> p b cl2 j", b=kgrp, cl2=ncl_mm
                            )
                            nc.scalar.copy(t3, pv)
                            nc.gpsimd.tensor_add(qv, qv, t3)
                        else:
                            nc.vector.tensor_add(qv, qv, pv)
                # store slot in pieces (cl ranges), contiguous in DRAM
                ncl_st = CL // stores_per_slot
                for s in range(stores_per_slot):
                    sb = qt_cells(m, s * ncl_st, ncl_st, 0, Js)
                    a = m * SW + s * ncl_st * Js
                    dr = y[:, a : a + ncl_st * Js].rearrange(
                        "p (cl j) -> p cl j", cl=ncl_st
                    )
                    nc.sync.dma_start(dr, sb)

    nc.compile()  # bacc passes: reg alloc, wait splitting (1-wait HW limit), ...
    return nc


def _consts():
    # sel4[u*4+jj', 32g+t] for block jj: 1 iff jj'==jj and u==t
    sel = np.zeros((C, 4 * C), np.float32)
    for jj in range(4):
        for t in range(T):
            for g in range(G):
                sel[t * 4 + jj, jj * C + g * 32 + t] = 1.0
    id32 = np.zeros((C, T), np.float32)
    for p in range(C):
        id32[p, p % T] = 1.0
    return sel, id32


_BUILD_KW = dict(mode="exact")


_NSLOT = 4  # must match build_nc(nslot=...)


def make_in_maps(x: np.ndarray, alpha: np.ndarray):
    assert x.shape == (N, C, T, H, W) and x.dtype == np.float32
    sel, id32 = _consts()
    alpha_rep = np.full((C, 1), np.float32(alpha.reshape(-1)[0]), np.float32)
    # slot-major stripe: x_str[p, m*T*Js + t*Js + j] = x[p, t, m*Js + j]
    Js = HB // _NSLOT
    xr = np.ascontiguousarray(
        x.reshape(N, C, T, _NSLOT, Js).transpose(0, 1, 3, 2, 4).reshape(N, C, F)
    )
    return [
        {"x": xr[n], "alpha_rep": alpha_rep, "sel4": sel, "ident32": id32}
        for n in range(NCORES)
    ]


def kernel(x: np.ndarray, alpha: np.ndarray) -> np.ndarray:
    from concourse.bass_utils import run_bass_kernel_spmd

    nc = build_nc(**_BUILD_KW)
    in_maps = make_in_maps(x, alpha)
    res = run_bass_kernel_spmd(nc, in_maps, list(range(NCORES)))
    out = np.stack([unfold_y(res.results[n]["y"]) for n in range(NCORES)])
    return out.astype(np.float32)


def unfold_y(yf: np.ndarray) -> np.ndarray:
    # yf[32g+t, m*SW + cl*Js + jj] = out[32g+cl, t, m*Js+jj]  ->  (C, T, H, W)
    Js = HB // _NSLOT
    return (
        np.asarray(yf)
        .reshape(G, T, _NSLOT, CL, Js)
        .transpose(0, 3, 1, 2, 4)
        .reshape(C, T, H, W)
    )
